# revision 1
# baseline (speedup 1.0000x reference)
"""GQA attention layer (QKV proj + RoPE + causal GQA attention + o_proj) on 8 trn2
NeuronCores.

Sharding: DP=2 over batch x TP=4 over heads (8 Q heads / 2 KV heads per core).
Per-core: QKV projection in transposed layout (features on partitions, fp32r
matmuls), RoPE via host-interleaved pair layout + DVE stream_shuffle, flash-style
causal attention with transposed scores (exp on ACT over 2-ks-tile PSUM groups,
bf16 p/V, softmax sums via ones-matmuls, binary causal masks applied post-exp),
a 3-chunk uneven AllGather overlapped with the attention tail, and a
column-sharded fp32r o_proj.  Host reassembles [B, S, H] from per-core column
shards.
"""

import ml_dtypes
import numpy as np

import concourse.bass as bass
import concourse.bacc as bacc
import concourse.tile as tile
import concourse.mybir as mybir
from concourse import bass_utils

F32 = mybir.dt.float32
F32R = mybir.dt.float32r

# Model shape (hardcoded for nn_Attention_38147899523668)
B, S, H = 2, 2048, 2048
NH, NKV, HD = 32, 8, 64
G = NH // NKV
SCALE = 1.0 / np.sqrt(HD)

# Sharding
N_CORES = 8
TP = 4                    # tensor-parallel group size (heads)
DP = N_CORES // TP        # data-parallel over batch
NH_L = NH // TP           # 8 Q heads per core
NKV_L = NKV // TP         # 2 KV heads per core
DQ = NH_L * HD            # 512 rows of q per core
DKV = NKV_L * HD          # 128 rows of k/v per core

SBLK = 512                # qs block (also matmul N)
EXPSPLIT = False          # exp per ks-tile instead of per group
SWAPMASK = [i ^ 1 for i in range(32)]   # adjacent-pair swap (RoPE interleaved)
KT = 128                  # ks tile
EXPG = 2                  # ks tiles per exp group


def build_program(nc, s=S, tp_groups=None, fake_gather=False, n_ag=3):
    """Emit the per-core SPMD program. Same NEFF on all cores; all per-core
    differences come through the input tensors."""
    if tp_groups is None:
        tp_groups = [[0, 1, 2, 3], [4, 5, 6, 7]]
    tp = len(tp_groups[0])
    n_sb = s // SBLK          # qs blocks
    n_ht = H // 128           # h (contraction) tiles for projections
    n_dt = DQ // 128          # q d-tiles (4)
    n_st = s // 128           # s tiles (for v transpose / o_proj)
    n_ft = tp * n_dt          # feature tiles of gathered attention (16)
    EO = DQ                   # output columns per core (o_proj col shard)
    n_diag = SBLK // KT       # diagonal ks tiles per qs block (4)
    if n_sb >= 4 and n_ag == 3:
        chunks = [(0, n_sb - 2), (n_sb - 2, n_sb - 1), (n_sb - 1, n_sb)]
    elif n_sb >= 4 and n_ag == 2:
        chunks = [(0, n_sb - 1), (n_sb - 1, n_sb)]
    else:
        chunks = [(0, n_sb)]
    n_ck = len(chunks)
    ck_of_qi = {}
    for ci, (a, b_) in enumerate(chunks):
        for q_ in range(a, b_):
            ck_of_qi[q_] = ci

    BF16 = mybir.dt.bfloat16
    inp = {}
    def din(name, shape, dtype=F32):
        inp[name] = nc.dram_tensor(name, shape, dtype, kind="ExternalInput").ap()
        return inp[name]

    xT = din("xT", [H, s])
    wqT = din("wqT", [H, DQ])
    wkT = din("wkT", [H, DKV])
    wvT = din("wvT", [H, DKV])
    bq = din("bq", [128, n_dt])
    bk = din("bk", [128, 1])
    bv = din("bv", [128, 1])
    cos_rep = din("cos_rep", [128, s])
    sin_pm = din("sin_pm", [128, s])
    woT = din("woT", [H, EO], dtype=BF16)
    out = nc.dram_tensor("out", [s, EO], F32, kind="ExternalOutput").ap()

    with tile.TileContext(nc) as tc:
        with tc.tile_pool(name="dram", bufs=1, space="DRAM") as dram, \
             tc.tile_pool(name="ps2", bufs=2, space="PSUM") as ps2, \
             tc.tile_pool(name="ps1", bufs=4, space="PSUM") as ps1:
            ag_in = [dram.tile([n_dt, 128, (b_ - a) * SBLK], BF16,
                               tag=f"ag_in{ci}", name=f"ag_in{ci}")
                     for ci, (a, b_) in enumerate(chunks)]
            if tp > 1:
                ag_out = [dram.tile([tp, n_dt, 128, (b_ - a) * SBLK], BF16,
                                    tag=f"ag_out{ci}", name=f"ag_out{ci}")
                          for ci, (a, b_) in enumerate(chunks)]

            # ============ phases 1+2 scope ============
            with tc.tile_pool(name="acts", bufs=1) as acts, \
                 tc.tile_pool(name="consts", bufs=1) as consts, \
                 tc.tile_pool(name="pT", bufs=8) as pT_p, \
                 tc.tile_pool(name="nrm", bufs=2) as nrm_p:
                qrot = acts.tile([128, n_dt, s], F32, tag="qrot")
                kT_rep = acts.tile([128, NKV_L, s], F32R, tag="kT_rep")
                v_sb = acts.tile([128, n_st, NKV_L * HD], BF16, tag="v_sb")

                ones16 = consts.tile([128, 64], BF16, tag="ones16")
                nc.vector.memset(ones16[:], 1.0)
                cos_sb = consts.tile([128, s], F32, tag="cos")
                sin_sb = consts.tile([128, s], F32, tag="sin")
                bq_sb = consts.tile([128, n_dt], F32, tag="bq")
                bk_sb = consts.tile([128, 1], F32, tag="bk")
                bv_sb = consts.tile([128, 1], F32, tag="bv")

                # binary causal mask for the diagonal ks tiles of a qs block:
                # maskc[p, m, q] = 1.0 if q >= KT*m + p else 0.0
                maskc = consts.tile([128, n_diag, SBLK], BF16, tag="maskc")
                nc.gpsimd.memset(maskc[:], 1.0)
                nc.gpsimd.affine_select(
                    out=maskc[:], in_=maskc[:],
                    compare_op=mybir.AluOpType.is_ge,
                    fill=0.0, base=0,
                    pattern=[[-KT, n_diag], [1, SBLK]],
                    channel_multiplier=-1,
                )

                ident = consts.tile([128, 128], F32, tag="ident")
                nc.gpsimd.memset(ident[:], 0.0)
                nc.gpsimd.affine_select(
                    out=ident[:], in_=ident[:],
                    compare_op=mybir.AluOpType.not_equal,
                    fill=1.0, base=0,
                    pattern=[[-1, 128]], channel_multiplier=1,
                )

                # ---- causal GQA attention for one qs block (emitted inline
                # after that block's QKV epilogue so exp/PE overlap QKV) ----
                def emit_attention(qi):
                    qs = qi * SBLK
                    ck = ck_of_qi[qi]
                    qoff = (qi - chunks[ck][0]) * SBLK
                    nk = (qi + 1) * n_diag        # ks tiles (causal)
                    dt0 = nk - n_diag             # first diagonal ks tile
                    for j in range(NKV_L):
                        for pr in range(2):       # head pair = q d-tile
                            dt = 2 * j + pr
                            ctx = ps1.tile([128, SBLK], F32, tag="b1")
                            sums = ps1.tile([128, SBLK], F32, tag="b1")
                            ngrp = (nk + EXPG - 1) // EXPG
                            for g in range(ngrp):
                                t0 = g * EXPG
                                lastg = g == ngrp - 1
                                # per-tile valid-qs offset (causal): tiles m>=2
                                # of the diagonal band only touch qs >= 128*m
                                offs = []
                                for tt in range(EXPG):
                                    m = t0 + tt - dt0
                                    offs.append(KT * m if (lastg and m >= 2) else 0)
                                sc = [ps2.tile([128, EXPG, SBLK], F32,
                                               tag="b2", name=f"sc{h_}")
                                      for h_ in range(2)]
                                for tt in range(EXPG):
                                    ks = (t0 + tt) * KT
                                    # fp32r matmul needs N >= 256
                                    soff = min(offs[tt], SBLK - 256)
                                    for h in range(2):   # row-packed pair
                                        hb = h * 64
                                        nc.tensor.matmul(
                                            sc[h][:, tt, soff:],
                                            kT_rep[hb:hb + 64, j, ks:ks + KT],
                                            qrot[hb:hb + 64, dt,
                                                 qs + soff:qs + SBLK].bitcast(F32R),
                                            start=True, stop=True)
                                pt = [pT_p.tile([128, EXPG, SBLK], BF16,
                                                tag="pt", name=f"pt{h_}")
                                      for h_ in range(2)]
                                for h in range(2):
                                    if lastg:
                                        for tt in range(EXPG):
                                            nc.scalar.activation(
                                                out=pt[h][:, tt, offs[tt]:],
                                                in_=sc[h][:, tt, offs[tt]:],
                                                func=mybir.ActivationFunctionType.Exp,
                                                scale=SCALE)
                                    else:
                                        nc.scalar.activation(
                                            out=pt[h][:], in_=sc[h][:],
                                            func=mybir.ActivationFunctionType.Exp,
                                            scale=SCALE)
                                for h in range(2):
                                    for tt in range(EXPG):
                                        m = t0 + tt - dt0
                                        if m >= 0:
                                            nc.vector.tensor_mul(
                                                pt[h][:, tt, offs[tt]:],
                                                pt[h][:, tt, offs[tt]:],
                                                maskc[:, m, offs[tt]:])
                                for tt in range(EXPG):
                                    kt_i = t0 + tt
                                    first, last = kt_i == 0, kt_i == nk - 1
                                    off = offs[tt]
                                    # AV pair then sums pair: adjacent MMs hit
                                    # different PE column groups (0-63/64-127)
                                    # so each pair runs col-packed
                                    for h in range(2):
                                        nc.tensor.matmul(
                                            ctx[h * 64:h * 64 + 64, off:],
                                            v_sb[:, kt_i, j * 64:j * 64 + 64],
                                            pt[h][:, tt, off:],
                                            start=first, stop=last,
                                            skip_group_check=True)
                                    for h in range(2):
                                        nc.tensor.matmul(
                                            sums[h * 64:h * 64 + 64, off:],
                                            ones16[:],
                                            pt[h][:, tt, off:],
                                            start=first, stop=last,
                                            skip_group_check=True)
                            recip = nrm_p.tile([128, SBLK], F32, tag="recip")
                            nc.vector.reciprocal(recip[:], sums[:])
                            anrm = nrm_p.tile([128, SBLK], BF16, tag="anrm")
                            nc.vector.tensor_mul(anrm[:], ctx[:], recip[:])
                            nc.scalar.dma_start(
                                out=ag_in[ck][dt, :, qoff:qoff + SBLK],
                                in_=anrm[:])


                # ---------------- phase 1: QKV projection + RoPE ----------------
                with tc.tile_pool(name="w1", bufs=1) as w1, \
                     tc.tile_pool(name="xs", bufs=4) as xs, \
                     tc.tile_pool(name="rope_tmp", bufs=2) as rtmp, \
                     tc.tile_pool(name="vt_tmp", bufs=2) as vtmp:

                    wq_sb = w1.tile([128, n_ht, DQ], F32R, tag="wq")
                    wk_sb = w1.tile([128, n_ht, DKV], F32R, tag="wk")
                    wv_sb = w1.tile([128, n_ht, DKV], F32R, tag="wv")
                    wq_c = wqT.rearrange("(c g p) d -> c p g d", p=128, g=4)
                    wk_t = wkT.rearrange("(t p) d -> t p d", p=128)
                    wv_t = wvT.rearrange("(t p) d -> t p d", p=128)
                    nc.sync.dma_start(out=wq_sb[:, 0:1, :],
                                      in_=wq_c[0][:, 0:1, :].bitcast(F32R))
                    nc.scalar.dma_start(out=wk_sb[:, 0, :],
                                        in_=wk_t[0].bitcast(F32R))
                    nc.scalar.dma_start(out=wv_sb[:, 0, :],
                                        in_=wv_t[0].bitcast(F32R))
                    nc.sync.dma_start(out=wq_sb[:, 1:4, :],
                                      in_=wq_c[0][:, 1:4, :].bitcast(F32R))
                    nc.scalar.dma_start(
                        out=wk_sb[:, 1:, :], in_=wk_t[1:].rearrange(
                            "t p d -> p t d").bitcast(F32R))
                    nc.scalar.dma_start(
                        out=wv_sb[:, 1:, :], in_=wv_t[1:].rearrange(
                            "t p d -> p t d").bitcast(F32R))
                    for c_ in range(1, n_ht // 4):
                        nc.scalar.dma_start(
                            out=wq_sb[:, 4 * c_:4 * (c_ + 1), :],
                            in_=wq_c[c_].bitcast(F32R))
                    nc.scalar.dma_start(out=bq_sb[:], in_=bq[:])
                    nc.scalar.dma_start(out=bk_sb[:], in_=bk[:])
                    nc.scalar.dma_start(out=bv_sb[:], in_=bv[:])
                    nc.scalar.dma_start(out=cos_sb[:], in_=cos_rep[:])
                    nc.scalar.dma_start(out=sin_sb[:], in_=sin_pm[:])

                    xT_c = xT.rearrange("(c g p) s -> c p g s", p=128, g=4)

                    for sb in range(n_sb):
                        ss = sb * SBLK
                        psq01 = ps2.tile([128, 2, SBLK], F32, tag="b2")
                        psq23 = ps2.tile([128, 2, SBLK], F32, tag="b2")
                        psq = [psq01, psq23]
                        psk = ps1.tile([128, SBLK], F32, tag="b1")
                        psv = ps1.tile([128, SBLK], F32, tag="b1")
                        for hc in range(n_ht // 4):
                            xt = xs.tile([128, 4, SBLK], F32R, tag="xt")
                            nc.sync.dma_start(
                                out=xt[:],
                                in_=xT_c[hc, :, :, ss:ss + SBLK].bitcast(F32R))
                            for hg in range(4):
                                ht = 4 * hc + hg
                                st0, st1 = (ht == 0), (ht == n_ht - 1)
                                for dt in range(n_dt):
                                    nc.tensor.matmul(
                                        psq[dt // 2][:, dt % 2, :],
                                        wq_sb[:, ht, dt * 128:(dt + 1) * 128],
                                        xt[:, hg, :], start=st0, stop=st1,
                                        skip_group_check=True)
                                nc.tensor.matmul(psk[:], wk_sb[:, ht, :],
                                                 xt[:, hg, :],
                                                 start=st0, stop=st1,
                                                 skip_group_check=True)
                                nc.tensor.matmul(psv[:], wv_sb[:, ht, :],
                                                 xt[:, hg, :],
                                                 start=st0, stop=st1,
                                                 skip_group_check=True)

                        # q RoPE epilogue
                        for dt in range(n_dt):
                            qraw = rtmp.tile([128, SBLK], F32, tag="qraw")
                            nc.vector.tensor_scalar_add(
                                out=qraw[:], in0=psq[dt // 2][:, dt % 2, :],
                                scalar1=bq_sb[:, dt:dt + 1])
                            qsw = rtmp.tile([128, SBLK], F32, tag="qsw")
                            nc.vector.stream_shuffle(qsw[:], qraw[:], SWAPMASK)
                            qcos = rtmp.tile([128, SBLK], F32, tag="qcos")
                            nc.vector.tensor_mul(qcos[:], qraw[:],
                                                 cos_sb[:, ss:ss + SBLK])
                            nc.vector.tensor_mul(qsw[:], qsw[:],
                                                 sin_sb[:, ss:ss + SBLK])
                            nc.vector.tensor_add(
                                qrot[:, dt, ss:ss + SBLK].bitcast(F32R),
                                qcos[:], qsw[:])

                        # k RoPE epilogue -> kT_rep (both 64-halves = same kv head)
                        kraw = rtmp.tile([128, SBLK], F32, tag="kraw")
                        nc.vector.tensor_scalar_add(
                            out=kraw[:], in0=psk[:], scalar1=bk_sb[:, 0:1])
                        ksw = rtmp.tile([128, SBLK], F32, tag="ksw")
                        nc.vector.stream_shuffle(ksw[:], kraw[:], SWAPMASK)
                        krot = rtmp.tile([128, SBLK], F32, tag="krot")
                        nc.vector.tensor_mul(krot[:], kraw[:], cos_sb[:, ss:ss + SBLK])
                        nc.vector.tensor_mul(ksw[:], ksw[:], sin_sb[:, ss:ss + SBLK])
                        nc.vector.tensor_add(krot[:], krot[:], ksw[:])
                        for j in range(NKV_L):
                            for half in range(2):
                                nc.scalar.dma_start(
                                    out=kT_rep[half * 64:half * 64 + 64, j,
                                               ss:ss + SBLK],
                                    in_=krot[j * 64:j * 64 + 64, :].bitcast(F32R))

                        # v epilogue: bias, then transpose to [s, hd] layout
                        vtb = vtmp.tile([128, SBLK], F32, tag="vtb")
                        nc.vector.tensor_scalar_add(
                            out=vtb[:], in0=psv[:], scalar1=bv_sb[:, 0:1])
                        for k in range(SBLK // 128):
                            st = sb * (SBLK // 128) + k
                            ptp = ps1.tile([128, 128], F32, tag="b1",
                                           padded_shape=[128, SBLK])
                            nc.tensor.transpose(
                                ptp[:], vtb[:, k * 128:(k + 1) * 128], ident[:])
                            nc.vector.tensor_copy(v_sb[:, st, :], ptp[:])

                # ---------------- phase 2: attention over all qs blocks ----------
                for qi in range(n_sb):
                    emit_attention(qi)

                # ============ phase 3: AllGather + o_proj ============
                gathered = []
                for ci in range(n_ck):
                    if tp > 1 and fake_gather:
                        gathered.append(ag_out[ci][:].rearrange("r d p s -> (r d) p s"))
                    elif tp > 1:
                        nc.gpsimd.collective_compute(
                            "AllGather", mybir.AluOpType.bypass,
                            replica_groups=tp_groups,
                            ins=[ag_in[ci][:].opt()], outs=[ag_out[ci][:].opt()])
                        gathered.append(ag_out[ci][:].rearrange("r d p s -> (r d) p s"))
                    else:
                        gathered.append(ag_in[ci][:])

                with tc.tile_pool(name="w3", bufs=1) as w3, \
                     tc.tile_pool(name="afull", bufs=1) as afull_p, \
                     tc.tile_pool(name="osb", bufs=3) as osb_p:

                    woT_sb = w3.tile([128, n_ht, EO], BF16, tag="wo")
                    wo_c = woT.rearrange("(c g p) d -> c p g d", p=128, g=4)
                    for c in range(n_ht // 4):
                        nc.scalar.dma_start(out=woT_sb[:, 4 * c:4 * (c + 1), :],
                                            in_=wo_c[c])
                    afull = afull_p.tile([128, n_ft, s], BF16, tag="afull")
                    for ci, (a, b_) in enumerate(chunks):
                        eng = nc.sync if ci % 2 == 0 else nc.scalar
                        eng.dma_start(
                            out=afull[:, :, a * SBLK:b_ * SBLK],
                            in_=gathered[ci].rearrange("f p s -> p f s"))

                    for ci, (a, b_) in enumerate(chunks):
                        for sti in range(a * (SBLK // 128), b_ * (SBLK // 128)):
                            pso = ps1.tile([128, EO], F32, tag="b1")
                            for ft in range(n_ft):
                                nc.tensor.matmul(
                                    pso[:],
                                    afull[:, ft, sti * 128:(sti + 1) * 128],
                                    woT_sb[:, ft, :],
                                    start=(ft == 0), stop=(ft == n_ft - 1),
                                    skip_group_check=True)
                            ot = osb_p.tile([128, EO], F32, tag="ot")
                            nc.scalar.activation(
                                out=ot[:], in_=pso[:],
                                func=mybir.ActivationFunctionType.Copy, scale=1.0)
                            nc.sync.dma_start(
                                out=out[sti * 128:(sti + 1) * 128, :], in_=ot[:])


    return inp, out


def make_core_inputs(x, freqs_cos, freqs_sin, Wq, bq, Wk, bk, Wv, bv, Wo,
                     core, s=S, tp=TP):
    """Host-side shard/layout prep for one core."""
    b, r = core // tp, core % tp
    qh0 = r * NH_L              # first local Q head
    kh0 = r * NKV_L
    # head-dim pair interleave: new pos 2u <- old u, 2u+1 <- old u+32.  Scores
    # are invariant (same permutation on q and k); makes rotate-half an
    # adjacent-pair swap (DVE stream_shuffle) instead of a 32-block swap.
    idx = np.empty(HD, np.int64)
    idx[0::2] = np.arange(HD // 2)
    idx[1::2] = np.arange(HD // 2) + HD // 2
    xT = np.ascontiguousarray(x[b][:s].T).astype(np.float32)
    def permh(W, nh):
        W = W.reshape(nh, HD, -1)[:, idx, :]
        return W.reshape(nh * HD, -1)
    wq_l = permh(Wq[qh0 * HD:(qh0 + NH_L) * HD, :], NH_L)
    wk_l = permh(Wk[kh0 * HD:(kh0 + NKV_L) * HD, :], NKV_L)
    wqT = np.ascontiguousarray(wq_l.T)
    wkT = np.ascontiguousarray(wk_l.T)
    wvT = np.ascontiguousarray(Wv[kh0 * HD:(kh0 + NKV_L) * HD, :].T)
    bq_p = bq[qh0 * HD:(qh0 + NH_L) * HD].reshape(NH_L, HD)[:, idx].reshape(-1)
    bk_p = bk[kh0 * HD:(kh0 + NKV_L) * HD].reshape(NKV_L, HD)[:, idx].reshape(-1)
    bq_l = bq_p.reshape(NH_L // 2, 128).T
    bk_l = bk_p.reshape(1, 128).T
    bv_l = bv[kh0 * HD:(kh0 + NKV_L) * HD].reshape(1, 128).T
    # interleaved tables: cos_rep[p] = cos[:, (p%64)//2];
    # sin_pm[p] = (-1 if p even else +1) * sin[:, (p%64)//2]
    u = (np.arange(128) % HD) // 2
    cos_rep = freqs_cos[:s].T[u, :]
    sgn = np.where(np.arange(128) % 2 == 0, -1.0, 1.0)
    sin_pm = freqs_sin[:s].T[u, :] * sgn[:, None]
    # o_proj column shard: out[:, rDQ:(r+1)DQ] = attn_full @ Wo[rDQ:(r+1)DQ, :].T
    # rhs[k=f, N=e] needs Wo[e, f] with f on partitions -> woT = Wo_shard.T
    woT = np.ascontiguousarray(Wo[r * DQ:(r + 1) * DQ, :].T.astype(ml_dtypes.bfloat16))
    return {
        "xT": xT, "wqT": wqT.astype(np.float32), "wkT": wkT.astype(np.float32),
        "wvT": wvT.astype(np.float32),
        "bq": np.ascontiguousarray(bq_l, np.float32),
        "bk": np.ascontiguousarray(bk_l, np.float32),
        "bv": np.ascontiguousarray(bv_l, np.float32),
        "cos_rep": np.ascontiguousarray(cos_rep, np.float32),
        "sin_pm": np.ascontiguousarray(sin_pm, np.float32),
        "woT": woT,
    }


_CACHED_NC = None


def _get_nc():
    global _CACHED_NC
    if _CACHED_NC is None:
        nc = bacc.Bacc("TRN2", target_bir_lowering=False, debug=False,
                       num_devices=N_CORES)
        build_program(nc)
        nc.compile()
        _CACHED_NC = nc
    return _CACHED_NC


def kernel(x, freqs_cos, freqs_sin, mask, Wq, bq, Wk, bk, Wv, bv, Wo):
    x = np.asarray(x, np.float32)
    args = tuple(np.asarray(a, np.float32) for a in
                 (freqs_cos, freqs_sin, Wq, bq, Wk, bk, Wv, bv, Wo))
    nc = _get_nc()
    in_maps = [make_core_inputs(x, *args, core=c) for c in range(N_CORES)]
    res = bass_utils.run_bass_kernel_spmd(nc, in_maps, core_ids=list(range(N_CORES)))
    out = np.empty((B, S, H), np.float32)
    for c in range(N_CORES):
        b, r = c // TP, c % TP
        out[b][:, r * DQ:(r + 1) * DQ] = res.results[c]["out"]
    return out

